# revision 4
# baseline (speedup 1.0000x reference)
"""Trainium2 Bass kernel for the "Neuron" message-passing module.

Math (per batch row b):
    x[b, :]   = neuron_inputs[b, ::16]                  (128 gathered columns)
    total[b]  = x[b, :] @ w + bias + tanh(alpha * prev_state[b])
    new_state = leaky_relu(total, 0.01)
    output    = tanh(total)

Strategy: pure data parallel over the batch across 8 NeuronCores.
The host gathers the 128 needed columns (static stride-16 connectivity —
pure data movement) and hands each core a transposed (128, 8192) fp32
slab so the device DMA is perfectly contiguous per partition.  On the
device, the dot products run on the tensor engine as 64 matmuls per
core (stationary = 128x128 x^T tile, moving = w as a (128,1) column),
accumulating into a (128, 64) PSUM tile whose layout (partition p,
column j) = batch j*128+p.  The scalar engine applies tanh / bias, the
vector engine builds the leaky relu, and one DMA returns both outputs.
"""

import os

import numpy as np

import concourse.bacc as bacc
import concourse.bass as bass  # noqa: F401
import concourse.mybir as mybir
from concourse.bass_utils import run_bass_kernel_spmd
from concourse.tile import TileContext

F32 = mybir.dt.float32
AF = mybir.ActivationFunctionType
ALU = mybir.AluOpType

B = 65536  # batch
FAN_IN = 128  # gathered columns
COL_STRIDE = 16  # INCOMING = arange(128) * 16
N_CORES = 8
R = B // N_CORES  # rows per core = 8192
J = R // 128  # PSUM columns per core = 64
N_CHUNKS = 4  # x DMA chunks per core
CW = R // N_CHUNKS  # chunk width (batch columns) = 2048
NEG_SLOPE = 0.01

# Stash of the most recent BassKernelResults (exec_time_ns etc.) for the
# local test harness; harmless for grading.
LAST_RESULT = None


def _build_nc():
    nc = bacc.Bacc("TRN2", target_bir_lowering=False, num_devices=N_CORES)

    xt = nc.dram_tensor("xt", [FAN_IN, R], F32, kind="ExternalInput")
    pv = nc.dram_tensor("pv", [128, J], F32, kind="ExternalInput")
    # cst columns: 0 = weights (one per partition), 1 = alpha, 2 = bias
    cst = nc.dram_tensor("cst", [128, 4], F32, kind="ExternalInput")
    ob = nc.dram_tensor("ob", [128, 2 * J], F32, kind="ExternalOutput")

    with TileContext(nc) as tc:
        with (
            tc.tile_pool(name="pool", bufs=1) as pool,
            tc.tile_pool(name="pp", bufs=1, space="PSUM") as pp,
        ):
            x_sb = [
                pool.tile([FAN_IN, CW], F32, name=f"x{i}", tag=f"x{i}")
                for i in range(N_CHUNKS)
            ]
            for i in range(N_CHUNKS):
                nc.sync.dma_start(x_sb[i][:], xt[:, i * CW : (i + 1) * CW])

            cst_sb = pool.tile([128, 4], F32, tag="cst")
            pv_sb = pool.tile([128, J], F32, tag="pv")
            nc.scalar.dma_start(cst_sb[:], cst[:])
            nc.scalar.dma_start(pv_sb[:], pv[:])

            # tpb = tanh(alpha * prev) + bias
            t1 = pool.tile([128, J], F32, tag="t1")
            nc.scalar.activation(t1[:], pv_sb[:], AF.Tanh, scale=cst_sb[:, 1:2])
            tpb = pool.tile([128, J], F32, tag="tpb")
            nc.scalar.activation(tpb[:], t1[:], AF.Identity, bias=cst_sb[:, 2:3])

            # 64 dots of 128 batches each: ps[p, j] = x[j*128+p, :] @ w
            ps = pp.tile([128, J], F32)
            tiles_per_chunk = CW // 128
            for j in range(J):
                ci, lo = divmod(j, tiles_per_chunk)
                nc.tensor.matmul(
                    ps[:, j : j + 1],
                    x_sb[ci][:, lo * 128 : (lo + 1) * 128],
                    cst_sb[:, 0:1],
                    start=True,
                    stop=True,
                )

            tot = pool.tile([128, J], F32, tag="tot")
            nc.vector.tensor_tensor(tot[:], ps[:], tpb[:], op=ALU.add)

            ob_sb = pool.tile([128, 2 * J], F32, tag="ob")
            # new_state = max(total, 0.01 * total)
            nc.vector.tensor_scalar_mul(ob_sb[:, 0:J], tot[:], NEG_SLOPE)
            nc.vector.tensor_tensor(ob_sb[:, 0:J], ob_sb[:, 0:J], tot[:], op=ALU.max)
            # output = tanh(total)
            nc.scalar.activation(ob_sb[:, J : 2 * J], tot[:], AF.Tanh)

            nc.sync.dma_start(ob[:], ob_sb[:])

    nc.compile()
    return nc


_NC = None


def _get_nc():
    global _NC
    if _NC is None:
        _NC = _build_nc()
    return _NC


def kernel(model_inputs, neuron_inputs, prev_state, weights, bias, alpha):
    global LAST_RESULT
    neuron_inputs = np.asarray(neuron_inputs, dtype=np.float32)
    prev_state = np.asarray(prev_state, dtype=np.float32)
    weights = np.asarray(weights, dtype=np.float32)
    bias = np.asarray(bias, dtype=np.float32)
    alpha = np.asarray(alpha, dtype=np.float32)

    # Host-side shard prep (data movement only).  Gather the 128 used
    # columns with a single streaming pass, then per-core transpose so
    # each device DMA is contiguous per partition.
    xg = np.ascontiguousarray(neuron_inputs[:, ::COL_STRIDE])  # (B, 128)
    cst = np.zeros((128, 4), dtype=np.float32)
    cst[:, 0] = weights
    cst[:, 1] = np.float32(alpha)
    cst[:, 2] = np.float32(bias.reshape(-1)[0])
    pv3 = np.ascontiguousarray(
        prev_state.reshape(N_CORES, J, 128).transpose(0, 2, 1)
    )  # (cores, p, j)

    in_maps = []
    for c in range(N_CORES):
        xt_c = np.ascontiguousarray(xg[c * R : (c + 1) * R, :].T)  # (128, R)
        in_maps.append({"xt": xt_c, "pv": pv3[c], "cst": cst})

    trace = os.environ.get("BASS_KERNEL_TRACE", "0") == "1"
    if trace:
        try:
            from antenv.axon_hooks import get_axon_ntff_profile_hook  # noqa: F401
        except ImportError:
            trace = False
    res = run_bass_kernel_spmd(
        _get_nc(), in_maps, core_ids=list(range(N_CORES)), trace=trace
    )
    LAST_RESULT = res

    outs = np.stack([r["ob"] for r in res.results])  # (cores, 128, 2J)
    new_state = outs[:, :, :J].transpose(0, 2, 1).reshape(B)
    output = outs[:, :, J:].transpose(0, 2, 1).reshape(B)
    return new_state, output


# revision 12
# speedup vs baseline: 1.2343x; 1.2343x over previous
"""Trainium2 Bass kernel for the "Neuron" message-passing module.

Math (per batch row b):
    x[b, :]   = neuron_inputs[b, ::16]                  (128 gathered columns)
    total[b]  = x[b, :] @ w + bias + tanh(alpha * prev_state[b])
    new_state = leaky_relu(total, 0.01)
    output    = tanh(total)

Strategy: pure data parallel over the batch across 8 NeuronCores; the
host gathers the 128 needed columns (static stride-16 connectivity —
pure data movement) and ships each core a (8192, 128) fp32 slab in the
natural row layout.  On the device (raw bacc, hand-placed semaphores —
no Tile pre/post barriers):
  - SP streams the slab into SBUF in 8 chunks (HWDGE, 512B descriptors).
  - DVE computes the 64 dot products per 128-batch tile with the fused
    affine_mul_reduce custom op (multiply by a broadcast w tile +
    reduce-add),
    producing total in a compact (128, 64) layout where (partition p,
    column j) = batch j*128 + p.  It then adds tanh(alpha*prev)+bias and
    builds the leaky relu.
  - ACT computes tanh(alpha*prev)+bias up front and tanh(total) at the
    end.
  - SP DMAs the (128, 128) [new_state | output] tile back.
"""

import os
from contextlib import ExitStack

import numpy as np

import concourse.bacc as bacc
import concourse.mybir as mybir
from concourse.bass_utils import run_bass_kernel_spmd

F32 = mybir.dt.float32
AF = mybir.ActivationFunctionType
ALU = mybir.AluOpType

B = 65536  # batch
FAN_IN = 128  # gathered columns
COL_STRIDE = 16  # INCOMING = arange(128) * 16
N_CORES = 8
R = B // N_CORES  # rows per core = 8192
J = R // 128  # output columns per core = 64
N_CHUNKS = 8  # x DMA chunks per core
TPC = J // N_CHUNKS  # 128-row tiles per chunk = 8
CW = R // N_CHUNKS  # rows per chunk = 1024
NEG_SLOPE = 0.01

LAST_RESULT = None


def _build_nc():
    nc = bacc.Bacc("TRN2", target_bir_lowering=False, num_devices=N_CORES)

    xt = nc.dram_tensor("xt", [R, FAN_IN], F32, kind="ExternalInput")
    pv = nc.dram_tensor("pv", [128, J], F32, kind="ExternalInput")
    wb = nc.dram_tensor("wb", [128, FAN_IN], F32, kind="ExternalInput")
    # cst columns: 0 = alpha, 1 = bias (replicated down partitions)
    cst = nc.dram_tensor("cst", [128, 4], F32, kind="ExternalInput")
    ob = nc.dram_tensor("ob", [128, 2 * J], F32, kind="ExternalOutput")

    with (
        nc.sbuf_tensor("x_sb", [128, R], F32) as x_sb,
        nc.sbuf_tensor("wb_sb", [128, FAN_IN], F32) as wb_sb,
        nc.sbuf_tensor("pv_sb", [128, J], F32) as pv_sb,
        nc.sbuf_tensor("cst_sb", [128, 4], F32) as cst_sb,
        nc.sbuf_tensor("tpb_sb", [128, J], F32) as tpb_sb,
        nc.sbuf_tensor("tot_sb", [128, J], F32) as tot_sb,
        nc.sbuf_tensor("tmp_sb", [128, J], F32) as tmp_sb,
        nc.sbuf_tensor("tmp2_sb", [128, J], F32) as tmp2_sb,
        nc.sbuf_tensor("scr_sb", [128, 8 * FAN_IN], F32) as scr_sb,
        nc.sbuf_tensor("ob_sb", [128, 2 * J], F32) as ob_sb,
        ExitStack() as _sems,
        nc.semaphore("asem") as asem,
        nc.semaphore("acp") as acp,
        nc.semaphore("dvp") as dvp,
        nc.semaphore("prep_sem") as prep_sem,
        nc.semaphore("tot_sem") as tot_sem,
        nc.semaphore("dve_out_sem") as dve_out_sem,
        nc.semaphore("act_out_sem") as act_out_sem,
        nc.semaphore("osem") as osem,
        nc.Block() as block,
    ):
        xsem = [
            _sems.enter_context(nc.semaphore(f"xs{i}")) for i in range(N_CHUNKS)
        ]
        x3 = x_sb.ap().rearrange("p (j c) -> p j c", c=FAN_IN)  # (128, J, 128)

        @block.sync
        def _(sync):
            for ci in range(N_CHUNKS):
                src = xt[ci * CW : (ci + 1) * CW, :].rearrange(
                    "(j p) c -> p j c", p=128
                )
                sync.dma_start(
                    x3[:, ci * TPC : (ci + 1) * TPC, :], src
                ).then_inc(xsem[ci], 16)
            sync.wait_ge(dve_out_sem, 1)
            sync.wait_ge(act_out_sem, 1)
            sync.dma_start(ob[:], ob_sb[:]).then_inc(osem, 16)
            sync.wait_ge(osem, 16)

        @block.scalar
        def _(scalar):
            scalar.dma_start(cst_sb[:], cst[:]).then_inc(asem, 16)
            scalar.dma_start(pv_sb[:], pv[:]).then_inc(asem, 16)
            scalar.dma_start(wb_sb[:], wb[:]).then_inc(asem, 16)
            scalar.wait_ge(asem, 48)
            # tpb = tanh(alpha * prev) + bias
            scalar.activation(tmp_sb[:], pv_sb[:], AF.Tanh, scale=cst_sb[:, 0:1]).then_inc(
                acp, 1
            )
            scalar.wait_ge(acp, 1)  # ACT pipeline: tmp write must land
            scalar.activation(
                tpb_sb[:], tmp_sb[:], AF.Identity, bias=cst_sb[:, 1:2]
            ).then_inc(prep_sem, 1)
            # output = tanh(total)
            scalar.wait_ge(tot_sem, 1)
            scalar.activation(ob_sb[:, J : 2 * J], tot_sb[:], AF.Tanh).then_inc(
                act_out_sem, 1
            )

        @block.vector
        def _(vector):
            vector.wait_ge(asem, 48)  # wb_sb ready
            for ci in range(N_CHUNKS):
                vector.wait_ge(xsem[ci], 16)
                for k in range(TPC):
                    j = ci * TPC + k
                    if j >= 8:
                        # scratch slice reuse: writes of op j-8 must land
                        vector.wait_ge(dvp, j - 7)
                    s = (j % 8) * FAN_IN
                    vector.affine_mul_reduce(
                        out=scr_sb[:, s : s + FAN_IN],
                        accum_out=tot_sb[:, j : j + 1],
                        in0=x3[:, j, :],
                        in1=wb_sb[:],
                        scale=1.0,
                        bias=0.0,
                    ).then_inc(dvp, 1)
            vector.wait_ge(prep_sem, 1)
            vector.wait_ge(dvp, J)  # all tot columns landed
            # total += tanh(alpha*prev) + bias
            vector.tensor_tensor(tot_sb[:], tot_sb[:], tpb_sb[:], op=ALU.add).then_inc(
                tot_sem, 1
            )
            vector.wait_ge(tot_sem, 1)  # total writes landed
            # new_state = max(total, 0.01 * total)
            vector.tensor_scalar_mul(tmp2_sb[:], tot_sb[:], NEG_SLOPE).then_inc(dvp, 1)
            vector.wait_ge(dvp, J + 1)
            vector.tensor_tensor(
                ob_sb[:, 0:J], tmp2_sb[:], tot_sb[:], op=ALU.max
            ).then_inc(dve_out_sem, 1)

    nc.compile()
    return nc


_NC = None


def _get_nc():
    global _NC
    if _NC is None:
        _NC = _build_nc()
    return _NC


def kernel(model_inputs, neuron_inputs, prev_state, weights, bias, alpha):
    global LAST_RESULT
    neuron_inputs = np.asarray(neuron_inputs, dtype=np.float32)
    prev_state = np.asarray(prev_state, dtype=np.float32)
    weights = np.asarray(weights, dtype=np.float32)
    bias = np.asarray(bias, dtype=np.float32)
    alpha = np.asarray(alpha, dtype=np.float32)

    # Host-side shard prep (data movement only): one streaming pass
    # gathers the 128 used columns; row-range views shard the batch.
    xg = np.ascontiguousarray(neuron_inputs[:, ::COL_STRIDE])  # (B, 128)
    wbt = np.broadcast_to(weights, (128, FAN_IN)).copy()
    cst = np.zeros((128, 4), dtype=np.float32)
    cst[:, 0] = np.float32(alpha)
    cst[:, 1] = np.float32(bias.reshape(-1)[0])
    pv3 = np.ascontiguousarray(
        prev_state.reshape(N_CORES, J, 128).transpose(0, 2, 1)
    )  # (cores, p, j)

    in_maps = [
        {"xt": xg[c * R : (c + 1) * R], "pv": pv3[c], "wb": wbt, "cst": cst}
        for c in range(N_CORES)
    ]

    trace = os.environ.get("BASS_KERNEL_TRACE", "0") == "1"
    if trace:
        try:
            from antenv.axon_hooks import get_axon_ntff_profile_hook  # noqa: F401
        except ImportError:
            trace = False
    res = run_bass_kernel_spmd(
        _get_nc(), in_maps, core_ids=list(range(N_CORES)), trace=trace
    )
    LAST_RESULT = res

    outs = np.stack([r["ob"] for r in res.results])  # (cores, 128, 2J)
    new_state = outs[:, :, :J].transpose(0, 2, 1).reshape(B)
    output = outs[:, :, J:].transpose(0, 2, 1).reshape(B)
    return new_state, output


# revision 13
# speedup vs baseline: 1.4738x; 1.1941x over previous
"""Trainium2 Bass kernel for the "Neuron" message-passing module.

Math (per batch row b):
    x[b, :]   = neuron_inputs[b, ::16]                  (128 gathered columns)
    total[b]  = x[b, :] @ w + bias + tanh(alpha * prev_state[b])
    new_state = leaky_relu(total, 0.01)
    output    = tanh(total)

Strategy: pure data parallel over the batch across 8 NeuronCores; the
host gathers the 128 needed columns (static stride-16 connectivity —
pure data movement) and lays each core's slab out exactly as the SBUF
wants it: (128, 8192) where partition p, column j*128+c holds
x[j*128+p, c].  Device side (raw bacc, hand-placed semaphores — no Tile
framework):
  - SP streams wb then the slab in contiguous 2D chunks (HWDGE,
    4KB descriptors) and writes the result back at the end.
  - DVE computes the 64 dots per 128-batch tile with the fused
    affine_mul_reduce custom op, producing total in a compact (128, 64)
    layout ((partition p, column j) = batch j*128+p), then adds
    tanh(alpha*prev)+bias and builds the leaky relu.
  - ACT computes tanh(alpha*prev)+bias up front and tanh(total) at the
    end.
"""

import os

import numpy as np

import concourse.bacc as bacc
import concourse.mybir as mybir
from concourse.bass_utils import run_bass_kernel_spmd

F32 = mybir.dt.float32
AF = mybir.ActivationFunctionType
ALU = mybir.AluOpType

B = 65536  # batch
FAN_IN = 128  # gathered columns
COL_STRIDE = 16  # INCOMING = arange(128) * 16
N_CORES = 8
R = B // N_CORES  # rows per core = 8192
J = R // 128  # output columns per core = 64
N_CHUNKS = 16  # x DMA chunks per core
TPC = J // N_CHUNKS  # 128-row tiles per chunk
CW = R // N_CHUNKS  # elements per partition per chunk
NEG_SLOPE = 0.01

LAST_RESULT = None


def _build_nc():
    nc = bacc.Bacc("TRN2", target_bir_lowering=False, num_devices=N_CORES)

    xt = nc.dram_tensor("xt", [128, R], F32, kind="ExternalInput")
    pv = nc.dram_tensor("pv", [128, J], F32, kind="ExternalInput")
    wb = nc.dram_tensor("wb", [128, FAN_IN], F32, kind="ExternalInput")
    # cst columns: 0 = alpha, 1 = bias, 2/3 = zeros
    cst = nc.dram_tensor("cst", [128, 4], F32, kind="ExternalInput")
    ob = nc.dram_tensor("ob", [128, 2 * J], F32, kind="ExternalOutput")

    with (
        nc.sbuf_tensor("x_sb", [128, R], F32) as x_sb,
        nc.sbuf_tensor("wb_sb", [128, FAN_IN], F32) as wb_sb,
        nc.sbuf_tensor("pv_sb", [128, J], F32) as pv_sb,
        nc.sbuf_tensor("cst_sb", [128, 4], F32) as cst_sb,
        nc.sbuf_tensor("tpb_sb", [128, J], F32) as tpb_sb,
        nc.sbuf_tensor("tot_sb", [128, J], F32) as tot_sb,
        nc.sbuf_tensor("tmp_sb", [128, J], F32) as tmp_sb,
        nc.sbuf_tensor("tmp2_sb", [128, J], F32) as tmp2_sb,
        nc.sbuf_tensor("scr_sb", [128, 8 * FAN_IN], F32) as scr_sb,
        nc.sbuf_tensor("ob_sb", [128, 2 * J], F32) as ob_sb,
        nc.Block(no_gpsimd_drain=True) as block,
    ):
        wsem = nc.alloc_semaphore("wsem")
        asem = nc.alloc_semaphore("asem")
        acp = nc.alloc_semaphore("acp")
        dvp = nc.alloc_semaphore("dvp")
        prep_sem = nc.alloc_semaphore("prep_sem")
        tot_sem = nc.alloc_semaphore("tot_sem")
        dve_out_sem = nc.alloc_semaphore("dve_out_sem")
        act_out_sem = nc.alloc_semaphore("act_out_sem")
        osem = nc.alloc_semaphore("osem")
        xsem = [nc.alloc_semaphore(f"xs{i}") for i in range(N_CHUNKS)]
        x3 = x_sb.ap().rearrange("p (j c) -> p j c", c=FAN_IN)  # (128, J, 128)

        @block.sync
        def _(sync):
            sync.dma_start(wb_sb[:], wb[:]).then_inc(wsem, 16)
            for ci in range(N_CHUNKS):
                sync.dma_start(
                    x_sb[:, ci * CW : (ci + 1) * CW],
                    xt[:, ci * CW : (ci + 1) * CW],
                ).then_inc(xsem[ci], 16)
            sync.wait_ge(dve_out_sem, 1)
            sync.wait_ge(act_out_sem, 1)
            sync.dma_start(ob[:], ob_sb[:]).then_inc(osem, 16)
            sync.wait_ge(osem, 16)

        @block.scalar
        def _(scalar):
            scalar.dma_start(cst_sb[:], cst[:]).then_inc(asem, 16)
            scalar.dma_start(pv_sb[:], pv[:]).then_inc(asem, 16)
            scalar.wait_ge(asem, 32)
            # tpb = tanh(alpha * prev) + bias
            scalar.activation(
                tmp_sb[:],
                pv_sb[:],
                AF.Tanh,
                scale=cst_sb[:, 0:1],
                bias=cst_sb[:, 2:3],
            ).then_inc(acp, 1)
            scalar.wait_ge(acp, 1)  # ACT pipeline: tmp write must land
            scalar.activation(
                tpb_sb[:], tmp_sb[:], AF.Identity, bias=cst_sb[:, 1:2]
            ).then_inc(prep_sem, 1)
            # output = tanh(total)
            scalar.wait_ge(tot_sem, 1)
            scalar.activation(
                ob_sb[:, J : 2 * J], tot_sb[:], AF.Tanh, bias=cst_sb[:, 3:4]
            ).then_inc(act_out_sem, 1)

        @block.vector
        def _(vector):
            vector.wait_ge(wsem, 16)  # wb_sb ready
            for ci in range(N_CHUNKS):
                vector.wait_ge(xsem[ci], 16)
                for k in range(TPC):
                    j = ci * TPC + k
                    if j >= 8:
                        # scratch slice reuse: writes of op j-8 must land
                        vector.wait_ge(dvp, j - 7)
                    s = (j % 8) * FAN_IN
                    vector.affine_mul_reduce(
                        out=scr_sb[:, s : s + FAN_IN],
                        accum_out=tot_sb[:, j : j + 1],
                        in0=x3[:, j, :],
                        in1=wb_sb[:],
                        scale=1.0,
                        bias=0.0,
                    ).then_inc(dvp, 1)
            vector.wait_ge(prep_sem, 1)
            vector.wait_ge(dvp, J)  # all tot columns landed
            # total += tanh(alpha*prev) + bias
            vector.tensor_tensor(tot_sb[:], tot_sb[:], tpb_sb[:], op=ALU.add).then_inc(
                tot_sem, 1
            )
            vector.wait_ge(tot_sem, 1)  # total writes landed
            # new_state = max(total, 0.01 * total)
            vector.tensor_scalar_mul(tmp2_sb[:], tot_sb[:], NEG_SLOPE).then_inc(dvp, 1)
            vector.wait_ge(dvp, J + 1)
            vector.tensor_tensor(
                ob_sb[:, 0:J], tmp2_sb[:], tot_sb[:], op=ALU.max
            ).then_inc(dve_out_sem, 1)

    nc.compile()
    return nc


_NC = None


def _get_nc():
    global _NC
    if _NC is None:
        _NC = _build_nc()
    return _NC


def kernel(model_inputs, neuron_inputs, prev_state, weights, bias, alpha):
    global LAST_RESULT
    neuron_inputs = np.asarray(neuron_inputs, dtype=np.float32)
    prev_state = np.asarray(prev_state, dtype=np.float32)
    weights = np.asarray(weights, dtype=np.float32)
    bias = np.asarray(bias, dtype=np.float32)
    alpha = np.asarray(alpha, dtype=np.float32)

    # Host-side shard prep (data movement only): one streaming pass
    # gathers the 128 used columns, then a block transpose lays each
    # core's slab out in the on-chip layout (contiguous per partition).
    xg = np.ascontiguousarray(neuron_inputs[:, ::COL_STRIDE])  # (B, 128)
    xs = np.ascontiguousarray(
        xg.reshape(N_CORES, J, 128, FAN_IN).transpose(0, 2, 1, 3)
    ).reshape(N_CORES, 128, R)
    wbt = np.broadcast_to(weights, (128, FAN_IN)).copy()
    cst = np.zeros((128, 4), dtype=np.float32)
    cst[:, 0] = np.float32(alpha)
    cst[:, 1] = np.float32(bias.reshape(-1)[0])
    pv3 = np.ascontiguousarray(
        prev_state.reshape(N_CORES, J, 128).transpose(0, 2, 1)
    )  # (cores, p, j)

    in_maps = [
        {"xt": xs[c], "pv": pv3[c], "wb": wbt, "cst": cst} for c in range(N_CORES)
    ]

    trace = os.environ.get("BASS_KERNEL_TRACE", "0") == "1"
    if trace:
        try:
            from antenv.axon_hooks import get_axon_ntff_profile_hook  # noqa: F401
        except ImportError:
            trace = False
    res = run_bass_kernel_spmd(
        _get_nc(), in_maps, core_ids=list(range(N_CORES)), trace=trace
    )
    LAST_RESULT = res

    outs = np.stack([r["ob"] for r in res.results])  # (cores, 128, 2J)
    new_state = outs[:, :, :J].transpose(0, 2, 1).reshape(B)
    output = outs[:, :, J:].transpose(0, 2, 1).reshape(B)
    return new_state, output
